# revision 9
# baseline (speedup 1.0000x reference)
"""Chamfer distance loss kernel for Trainium2 (Bass/Tile), 8-core data parallel.

Problem: x, y [16, 2048, 3] fp32. Per batch b:
    P[i,j] = |x_i|^2 + |y_j|^2 - 2 x_i.y_j
    loss[b] = mean_j min_i P[i,j] + mean_i min_j P[i,j]

Strategy (v3):
  - Shard batch dim: 2 batches per core across 8 cores.
  - P = -2*Q with Q[i,j] = x_i.y_j - 0.5|x_i|^2 - 0.5|y_j|^2 as a K=13 bf16
    double-split augmented matmul.  min P == -2 * max Q.
  - PE row tiling: aug operands replicated at partitions 0/32/64/96; each
    m-tile issues 4 concurrent matmuls (tile_position=(32t,0)) covering its
    four 512-wide n-chunks -> one [128,2048] PSUM group, ~3x PE throughput.
  - ACT drains each group to bf16 SBUF (16 wide copies/batch); DVE runs the
    dl running-max chain at 2x and the dr row-max tree (level-batched).
  - dl partition-axis max via 16 XBAR DMA transposes (SP queue) + one
    strided reduce; final means via a DRAM-round-trip partition transpose
    (no PSUM needed outside the matmul groups).
"""

import sys

if "/opt/trn_rl_repo" not in sys.path:
    sys.path.insert(0, "/opt/trn_rl_repo")

import numpy as np

B, N, D = 16, 2048, 3
NCORES = 8
BPC = B // NCORES  # batches per core
MT = N // 128  # 16 m-tiles
Q = N // 128  # 16 points per partition in natural layout
K = 13

_CACHE = {}


def _build():
    from contextlib import ExitStack

    import concourse.bass as bass
    import concourse.mybir as mybir
    import concourse.tile as tile
    from concourse import bacc

    f32 = mybir.dt.float32
    bf16 = mybir.dt.bfloat16

    nc = bacc.Bacc()
    x = nc.dram_tensor("x", [BPC, N, D], f32, kind="ExternalInput")
    y = nc.dram_tensor("y", [BPC, N, D], f32, kind="ExternalInput")
    o = nc.dram_tensor("o", [1, BPC], f32, kind="ExternalOutput")

    X = mybir.AxisListType.X
    MAXOP = mybir.AluOpType.max

    with tile.TileContext(nc) as tc, ExitStack() as ctx:
        singles = ctx.enter_context(tc.tile_pool(name="singles", bufs=1))
        nat_pool = ctx.enter_context(tc.tile_pool(name="nat", bufs=2))
        stage_pool = ctx.enter_context(tc.tile_pool(name="stage", bufs=2))
        aug_pool = ctx.enter_context(tc.tile_pool(name="aug", bufs=2))
        small_pool = ctx.enter_context(tc.tile_pool(name="small", bufs=3))
        run_pool = ctx.enter_context(tc.tile_pool(name="run", bufs=2))
        dr_pool = ctx.enter_context(tc.tile_pool(name="dr", bufs=2))
        cp_pool = ctx.enter_context(tc.tile_pool(name="cp", bufs=3))
        mm_psum = ctx.enter_context(tc.tile_pool(name="mmps", bufs=2, space="PSUM"))

        out_sb = singles.tile([1, BPC], f32)
        tots = singles.tile([128, BPC], f32)
        scratch = nc.dram_tensor("scratch", [BPC, 2, K, 128, Q], bf16, kind="Internal")
        totdram = nc.dram_tensor("totdram", [BPC, 128], f32, kind="Internal")

        for b in range(BPC):
            # ---- setup: load both sides, split, flatten via DRAM ----
            natxy = nat_pool.tile([128, 2 * Q * D], f32, tag="natxy")
            nc.sync.dma_start(
                out=natxy[:, 0 : Q * D],
                in_=x[b].rearrange("(p q) d -> p (q d)", p=128),
            )
            nc.sync.dma_start(
                out=natxy[:, Q * D : 2 * Q * D],
                in_=y[b].rearrange("(p q) d -> p (q d)", p=128),
            )
            # [p, g, d, q] strided view
            natv = natxy.rearrange("p (g q d) -> p g d q", g=2, d=D)

            stxy = stage_pool.tile([128, 2 * K * Q], bf16, tag="stxy")
            stv = stxy.rearrange("p (g f q) -> p g f q", g=2, f=K)
            # x fields: [h0,h1,h2, h0,h1,h2, m0,m1,m2, nh,nm, 1,1]
            # y fields: [h0,h1,h2, m0,m1,m2, h0,h1,h2, 1,1, nh,nm]
            # h main: both sides fields 0-2 in one op
            nc.vector.tensor_copy(stv[:, :, 0:3, :], natv)
            # h dup: x fields 3-5, y fields 6-8 (ACT offload)
            nc.scalar.copy(stv[:, 0, 3:6, :], natv[:, 0])
            nc.scalar.copy(stv[:, 1, 6:9, :], natv[:, 1])
            # residual m = nat - h (both sides, one op)
            tmp = nat_pool.tile([128, 2 * Q * D], f32, tag="tmp")
            tmpv = tmp.rearrange("p (g d q) -> p g d q", g=2, q=Q)
            nc.vector.tensor_sub(tmpv, natv, stv[:, :, 0:3, :])
            nc.scalar.copy(stv[:, 0, 6:9, :], tmpv[:, 0])
            nc.scalar.copy(stv[:, 1, 3:6, :], tmpv[:, 1])
            # norms: -0.5*|.|^2, split h+m
            sq = nat_pool.tile([128, 2 * Q * D], f32, tag="sq")
            nc.vector.tensor_mul(sq, natxy, natxy)
            nrm = small_pool.tile([128, 2 * Q], f32, tag="nrm")
            nc.vector.tensor_reduce(
                nrm, sq.rearrange("p (g q d) -> p g q d", g=2, d=D), axis=X,
                op=mybir.AluOpType.add,
            )
            nc.vector.tensor_scalar_mul(nrm, nrm, -0.5)
            nrmv = nrm.rearrange("p (g q) -> p g q", g=2)
            nhx, nhy = stv[:, 0, 9, :], stv[:, 1, 11, :]
            nc.scalar.copy(nhx, nrmv[:, 0])
            nc.scalar.copy(nhy, nrmv[:, 1])
            nrm2 = small_pool.tile([128, 2 * Q], f32, tag="nrm2")
            nrm2v = nrm2.rearrange("p (g q) -> p g q", g=2)
            nc.vector.tensor_sub(nrm2v[:, 0], nrmv[:, 0], nhx)
            nc.vector.tensor_sub(nrm2v[:, 1], nrmv[:, 1], nhy)
            nc.scalar.copy(stv[:, 0, 10, :], nrm2v[:, 0])
            nc.scalar.copy(stv[:, 1, 12, :], nrm2v[:, 1])
            # ones fields
            nc.gpsimd.memset(stxy[:, 11 * Q : 13 * Q], 1.0)
            nc.gpsimd.memset(stxy[:, (K + 9) * Q : (K + 11) * Q], 1.0)

            nc.sync.dma_start(
                out=scratch[b].rearrange("g f p q -> p g f q"),
                in_=stv,
            )
            # aug replicated at partition blocks 0/32/64/96 for PE row tiling
            aug = aug_pool.tile([128, 2 * N], bf16, tag="aug")
            for r in range(4):
                nc.sync.dma_start(
                    out=aug[32 * r : 32 * r + K].rearrange(
                        "f (g p q) -> f g p q", g=2, q=Q
                    ),
                    in_=scratch[b].rearrange("g f p q -> f g p q"),
                )

            # ---- main loop: 16 quads of 4 concurrent matmuls ----
            runmax = run_pool.tile([128, N], bf16, tag="runmax")
            drbuf = dr_pool.tile([128, MT * 1024], bf16, tag="drbuf")
            for m in range(MT):
                psg = mm_psum.tile([128, 2048], f32, tag="mm")
                for t in range(4):
                    blk = aug[32 * t : 32 * t + K]
                    nc.tensor.matmul(
                        psg[:, t * 512 : (t + 1) * 512],
                        lhsT=blk[:, m * 128 : (m + 1) * 128],
                        rhs=blk[:, N + t * 512 : N + (t + 1) * 512],
                        start=True,
                        stop=True,
                        tile_position=(32 * t, 0),
                    )
                cp = cp_pool.tile([128, 2048], bf16, tag="cp")
                nc.scalar.copy(cp, psg)
                if m == 0:
                    nc.vector.tensor_copy(runmax, cp)
                else:
                    nc.vector.tensor_max(runmax, runmax, cp)
                # dr level 1: fold column halves
                nc.vector.tensor_max(
                    drbuf[:, m * 1024 : (m + 1) * 1024],
                    cp[:, 0:1024],
                    cp[:, 1024:2048],
                )

            # ---- dr tree levels 2.., strided in-place across all m ----
            drv = drbuf.rearrange("p (m c) -> p m c", c=1024)
            w = 512
            while w >= 16:
                nc.vector.tensor_max(
                    drv[:, :, 0:w], drv[:, :, 0:w], drv[:, :, w : 2 * w]
                )
                w //= 2
            dr16 = small_pool.tile([128, MT], f32, tag="dr16")
            nc.vector.tensor_reduce(
                dr16, drv[:, :, 0:16], axis=X, op=MAXOP,
            )

            # ---- dl: partition-axis max via XBAR DMA transposes ----
            rmT = run_pool.tile([128, N], bf16, tag="rmT")
            for c in range(MT):
                nc.sync.dma_start(
                    out=rmT[:, c * 128 : (c + 1) * 128],
                    in_=runmax[:, c * 128 : (c + 1) * 128],
                    transpose=True,
                )
            dl16 = small_pool.tile([128, MT], f32, tag="dl16")
            nc.vector.tensor_reduce(
                dl16,
                rmT.rearrange("p (c f) -> p c f", f=128),
                axis=X,
                op=MAXOP,
            )

            # ---- per-batch totals [128,1] ----
            dlsum = small_pool.tile([128, 1], f32, tag="dlsum")
            drsum = small_pool.tile([128, 1], f32, tag="drsum")
            nc.vector.reduce_sum(dlsum, dl16, axis=X)
            nc.vector.reduce_sum(drsum, dr16, axis=X)
            nc.vector.tensor_add(tots[:, b : b + 1], dlsum, drsum)
            nc.sync.dma_start(out=totdram[b], in_=tots[:, b : b + 1])

        # ---- final: partition sum via DRAM transpose, then scale ----
        totT = singles.tile([1, BPC * 128], f32)
        nc.sync.dma_start(
            out=totT, in_=totdram.rearrange("b p -> (b p)")
        )
        psums = singles.tile([1, BPC], f32)
        nc.vector.tensor_reduce(
            psums, totT.rearrange("o (b p) -> o b p", b=BPC), axis=X,
            op=mybir.AluOpType.add,
        )
        nc.vector.tensor_scalar_mul(out_sb, psums, -2.0 / N)
        nc.gpsimd.dma_start(out=o[0:1, 0:BPC], in_=out_sb)

    nc.compile()
    return nc


def _get_nc():
    if "nc" not in _CACHE:
        _CACHE["nc"] = _build()
    return _CACHE["nc"]


def kernel(x: np.ndarray, y: np.ndarray) -> np.ndarray:
    from concourse.bass_utils import run_bass_kernel_spmd

    x = np.ascontiguousarray(np.asarray(x, dtype=np.float32))
    y = np.ascontiguousarray(np.asarray(y, dtype=np.float32))
    nc = _get_nc()
    in_maps = [
        {"x": x[c * BPC : (c + 1) * BPC], "y": y[c * BPC : (c + 1) * BPC]}
        for c in range(NCORES)
    ]
    res = run_bass_kernel_spmd(nc, in_maps, core_ids=list(range(NCORES)))
    return np.concatenate([r["o"].reshape(BPC) for r in res.results])


# revision 12
# speedup vs baseline: 1.3005x; 1.3005x over previous
"""Chamfer distance loss kernel for Trainium2 (Bass/Tile), 8-core data parallel.

Problem: x, y [16, 2048, 3] fp32. Per batch b:
    P[i,j] = |x_i|^2 + |y_j|^2 - 2 x_i.y_j
    loss[b] = mean_j min_i P[i,j] + mean_i min_j P[i,j]

Strategy (v3):
  - Shard batch dim: 2 batches per core across 8 cores.
  - P = -2*Q with Q[i,j] = x_i.y_j - 0.5|x_i|^2 - 0.5|y_j|^2 as a K=13 bf16
    double-split augmented matmul.  min P == -2 * max Q.
  - PE row tiling: aug operands replicated at partitions 0/32/64/96; each
    m-tile issues 4 concurrent matmuls (tile_position=(32t,0)) covering its
    four 512-wide n-chunks -> one [128,2048] PSUM group, ~3x PE throughput.
  - ACT drains each group to bf16 SBUF (16 wide copies/batch); DVE runs the
    dl running-max chain at 2x and the dr row-max tree (level-batched).
  - dl partition-axis max via 16 XBAR DMA transposes (SP queue) + one
    strided reduce; final means via a DRAM-round-trip partition transpose
    (no PSUM needed outside the matmul groups).
"""

import sys

if "/opt/trn_rl_repo" not in sys.path:
    sys.path.insert(0, "/opt/trn_rl_repo")

import numpy as np

B, N, D = 16, 2048, 3
NCORES = 8
BPC = B // NCORES  # batches per core
MT = N // 128  # 16 m-tiles
Q = N // 128  # 16 points per partition in natural layout
K = 13

_CACHE = {}


def _build():
    from contextlib import ExitStack

    import concourse.bass as bass
    import concourse.mybir as mybir
    import concourse.tile as tile
    from concourse import bacc

    f32 = mybir.dt.float32
    bf16 = mybir.dt.bfloat16

    nc = bacc.Bacc()
    x = nc.dram_tensor("x", [BPC, N, D], f32, kind="ExternalInput")
    y = nc.dram_tensor("y", [BPC, N, D], f32, kind="ExternalInput")
    o = nc.dram_tensor("o", [1, BPC], f32, kind="ExternalOutput")

    X = mybir.AxisListType.X
    MAXOP = mybir.AluOpType.max

    with tile.TileContext(nc) as tc, ExitStack() as ctx:
        singles = ctx.enter_context(tc.tile_pool(name="singles", bufs=1))
        nat_pool = ctx.enter_context(tc.tile_pool(name="nat", bufs=2))
        stage_pool = ctx.enter_context(tc.tile_pool(name="stage", bufs=2))
        aug_pool = ctx.enter_context(tc.tile_pool(name="aug", bufs=2))
        small_pool = ctx.enter_context(tc.tile_pool(name="small", bufs=3))
        run_pool = ctx.enter_context(tc.tile_pool(name="run", bufs=2))
        dr_pool = ctx.enter_context(tc.tile_pool(name="dr", bufs=2))
        cp_pool = ctx.enter_context(tc.tile_pool(name="cp", bufs=3))
        mm_psum = ctx.enter_context(tc.tile_pool(name="mmps", bufs=2, space="PSUM"))

        out_sb = singles.tile([1, BPC], f32)
        scratch = nc.dram_tensor("scratch", [BPC, 2, K, 128, Q], bf16, kind="Internal")

        for b in range(BPC):
            # ---- setup: load both sides, split, flatten via DRAM ----
            natxy = nat_pool.tile([128, 2 * Q * D], f32, tag="natxy")
            nc.sync.dma_start(
                out=natxy[:, 0 : Q * D],
                in_=x[b].rearrange("(p q) d -> p (q d)", p=128),
            )
            nc.sync.dma_start(
                out=natxy[:, Q * D : 2 * Q * D],
                in_=y[b].rearrange("(p q) d -> p (q d)", p=128),
            )
            # [p, g, d, q] strided view
            natv = natxy.rearrange("p (g q d) -> p g d q", g=2, d=D)

            stxy = stage_pool.tile([128, 2 * K * Q], bf16, tag="stxy")
            stv = stxy.rearrange("p (g f q) -> p g f q", g=2, f=K)
            # x fields: [h0,h1,h2, h0,h1,h2, m0,m1,m2, nh,nm, 1,1]
            # y fields: [h0,h1,h2, m0,m1,m2, h0,h1,h2, 1,1, nh,nm]
            # h main: both sides fields 0-2 in one op
            nc.vector.tensor_copy(stv[:, :, 0:3, :], natv)
            # h dup: x fields 3-5, y fields 6-8 (ACT offload)
            nc.scalar.copy(stv[:, 0, 3:6, :], natv[:, 0])
            nc.scalar.copy(stv[:, 1, 6:9, :], natv[:, 1])
            # residual m = nat - h (both sides, one op)
            tmp = nat_pool.tile([128, 2 * Q * D], f32, tag="tmp")
            tmpv = tmp.rearrange("p (g d q) -> p g d q", g=2, q=Q)
            nc.vector.tensor_sub(tmpv, natv, stv[:, :, 0:3, :])
            nc.scalar.copy(stv[:, 0, 6:9, :], tmpv[:, 0])
            nc.scalar.copy(stv[:, 1, 3:6, :], tmpv[:, 1])
            # norms: -0.5*|.|^2, split h+m
            sq = nat_pool.tile([128, 2 * Q * D], f32, tag="sq")
            nc.vector.tensor_mul(sq, natxy, natxy)
            nrm = small_pool.tile([128, 2 * Q], f32, tag="nrm")
            nc.vector.tensor_reduce(
                nrm, sq.rearrange("p (g q d) -> p g q d", g=2, d=D), axis=X,
                op=mybir.AluOpType.add,
            )
            nc.vector.tensor_scalar_mul(nrm, nrm, -0.5)
            nrmv = nrm.rearrange("p (g q) -> p g q", g=2)
            nhx, nhy = stv[:, 0, 9, :], stv[:, 1, 11, :]
            nc.scalar.copy(nhx, nrmv[:, 0])
            nc.scalar.copy(nhy, nrmv[:, 1])
            nrm2 = small_pool.tile([128, 2 * Q], f32, tag="nrm2")
            nrm2v = nrm2.rearrange("p (g q) -> p g q", g=2)
            nc.vector.tensor_sub(nrm2v[:, 0], nrmv[:, 0], nhx)
            nc.vector.tensor_sub(nrm2v[:, 1], nrmv[:, 1], nhy)
            nc.scalar.copy(stv[:, 0, 10, :], nrm2v[:, 0])
            nc.scalar.copy(stv[:, 1, 12, :], nrm2v[:, 1])
            # ones fields
            nc.gpsimd.memset(stxy[:, 11 * Q : 13 * Q], 1.0)
            nc.gpsimd.memset(stxy[:, (K + 9) * Q : (K + 11) * Q], 1.0)

            nc.sync.dma_start(
                out=scratch[b].rearrange("g f p q -> p g f q"),
                in_=stv,
            )
            # aug replicated at partition blocks 0/32/64/96 for PE row tiling
            aug = aug_pool.tile([128, 2 * N], bf16, tag="aug")
            for r in range(4):
                eng = nc.sync if r % 2 == 0 else nc.scalar
                eng.dma_start(
                    out=aug[32 * r : 32 * r + K].rearrange(
                        "f (g p q) -> f g p q", g=2, q=Q
                    ),
                    in_=scratch[b].rearrange("g f p q -> f g p q"),
                )

            # ---- main loop: 16 quads of 4 concurrent matmuls ----
            runmax = run_pool.tile([128, N], bf16, tag="runmax")
            drbuf = dr_pool.tile([128, MT * 1024], bf16, tag="drbuf")
            for m in range(MT):
                psg = mm_psum.tile([128, 2048], f32, tag="mm")
                for t in range(4):
                    blk = aug[32 * t : 32 * t + K]
                    nc.tensor.matmul(
                        psg[:, t * 512 : (t + 1) * 512],
                        lhsT=blk[:, m * 128 : (m + 1) * 128],
                        rhs=blk[:, N + t * 512 : N + (t + 1) * 512],
                        start=True,
                        stop=True,
                        tile_position=(32 * t, 0),
                    )
                cp = cp_pool.tile([128, 2048], bf16, tag="cp")
                nc.scalar.copy(cp, psg)
                if m == 0:
                    nc.vector.tensor_copy(runmax, cp)
                else:
                    nc.vector.tensor_max(runmax, runmax, cp)
                # dr level 1: fold column halves
                nc.vector.tensor_max(
                    drbuf[:, m * 1024 : (m + 1) * 1024],
                    cp[:, 0:1024],
                    cp[:, 1024:2048],
                )

            # ---- dr tree levels 2.., strided in-place across all m ----
            drv = drbuf.rearrange("p (m c) -> p m c", c=1024)
            w = 512
            while w >= 16:
                nc.vector.tensor_max(
                    drv[:, :, 0:w], drv[:, :, 0:w], drv[:, :, w : 2 * w]
                )
                w //= 2
            dr16 = small_pool.tile([128, MT], f32, tag="dr16")
            nc.vector.tensor_reduce(
                dr16, drv[:, :, 0:16], axis=X, op=MAXOP,
            )

            # ---- dl: partition-axis max on the idle gpsimd engine ----
            from concourse import bass_isa

            dlall = run_pool.tile([128, N], bf16, tag="dlall")
            nc.gpsimd.partition_all_reduce(
                dlall, runmax, channels=128, reduce_op=bass_isa.ReduceOp.max
            )
            dlsum = small_pool.tile([1, 1], f32, tag="dlsum")
            nc.vector.reduce_sum(dlsum, dlall[0:1, :], axis=X)

            # ---- dr partition sum: all-reduce add, then combine ----
            drsum = small_pool.tile([128, 1], f32, tag="drsum")
            nc.vector.reduce_sum(drsum, dr16, axis=X)
            drall = small_pool.tile([128, 1], f32, tag="drall")
            nc.gpsimd.partition_all_reduce(
                drall, drsum, channels=128, reduce_op=bass_isa.ReduceOp.add
            )
            tot = small_pool.tile([1, 1], f32, tag="tot")
            nc.vector.tensor_add(tot, dlsum, drall[0:1, :])
            nc.vector.tensor_scalar_mul(out_sb[0:1, b : b + 1], tot, -2.0 / N)

        nc.gpsimd.dma_start(out=o[0:1, 0:BPC], in_=out_sb)

    nc.compile()
    return nc


def _get_nc():
    if "nc" not in _CACHE:
        _CACHE["nc"] = _build()
    return _CACHE["nc"]


def kernel(x: np.ndarray, y: np.ndarray) -> np.ndarray:
    from concourse.bass_utils import run_bass_kernel_spmd

    x = np.ascontiguousarray(np.asarray(x, dtype=np.float32))
    y = np.ascontiguousarray(np.asarray(y, dtype=np.float32))
    nc = _get_nc()
    in_maps = [
        {"x": x[c * BPC : (c + 1) * BPC], "y": y[c * BPC : (c + 1) * BPC]}
        for c in range(NCORES)
    ]
    res = run_bass_kernel_spmd(nc, in_maps, core_ids=list(range(NCORES)))
    return np.concatenate([r["o"].reshape(BPC) for r in res.results])


# revision 16
# speedup vs baseline: 1.3218x; 1.0164x over previous
"""Chamfer distance loss kernel for Trainium2 (Bass/Tile), 8-core data parallel.

Problem: x, y [16, 2048, 3] fp32. Per batch b:
    P[i,j] = |x_i|^2 + |y_j|^2 - 2 x_i.y_j
    loss[b] = mean_j min_i P[i,j] + mean_i min_j P[i,j]

Strategy (v3):
  - Shard batch dim: 2 batches per core across 8 cores.
  - P = -2*Q with Q[i,j] = x_i.y_j - 0.5|x_i|^2 - 0.5|y_j|^2 as a K=13 bf16
    double-split augmented matmul.  min P == -2 * max Q.
  - PE row tiling: aug operands replicated at partitions 0/32/64/96; each
    m-tile issues 4 concurrent matmuls (tile_position=(32t,0)) covering its
    four 512-wide n-chunks -> one [128,2048] PSUM group, ~3x PE throughput.
  - ACT drains each group to bf16 SBUF (16 wide copies/batch); DVE runs the
    dl running-max chain at 2x and the dr row-max tree (level-batched).
  - dl partition-axis max via 16 XBAR DMA transposes (SP queue) + one
    strided reduce; final means via a DRAM-round-trip partition transpose
    (no PSUM needed outside the matmul groups).
"""

import sys

if "/opt/trn_rl_repo" not in sys.path:
    sys.path.insert(0, "/opt/trn_rl_repo")

import numpy as np

B, N, D = 16, 2048, 3
NCORES = 8
BPC = B // NCORES  # batches per core
MT = N // 128  # 16 m-tiles
Q = N // 128  # 16 points per partition in natural layout
K = 13

_CACHE = {}


def _build():
    from contextlib import ExitStack

    import concourse.bass as bass
    import concourse.mybir as mybir
    import concourse.tile as tile
    from concourse import bacc

    f32 = mybir.dt.float32
    bf16 = mybir.dt.bfloat16

    nc = bacc.Bacc()
    x = nc.dram_tensor("x", [BPC, N, D], f32, kind="ExternalInput")
    y = nc.dram_tensor("y", [BPC, N, D], f32, kind="ExternalInput")
    o = nc.dram_tensor("o", [1, BPC], f32, kind="ExternalOutput")

    X = mybir.AxisListType.X
    MAXOP = mybir.AluOpType.max

    with tile.TileContext(nc) as tc, ExitStack() as ctx:
        singles = ctx.enter_context(tc.tile_pool(name="singles", bufs=1))
        nat_pool = ctx.enter_context(tc.tile_pool(name="nat", bufs=2))
        stage_pool = ctx.enter_context(tc.tile_pool(name="stage", bufs=2))
        aug_pool = ctx.enter_context(tc.tile_pool(name="aug", bufs=2))
        small_pool = ctx.enter_context(tc.tile_pool(name="small", bufs=3))
        run_pool = ctx.enter_context(tc.tile_pool(name="run", bufs=2))
        dr_pool = ctx.enter_context(tc.tile_pool(name="dr", bufs=2))
        cp_pool = ctx.enter_context(tc.tile_pool(name="cp", bufs=3))
        mm_psum = ctx.enter_context(tc.tile_pool(name="mmps", bufs=2, space="PSUM"))

        out_sb = singles.tile([1, BPC], f32)
        scratch = nc.dram_tensor("scratch", [BPC, 2, K, 128, Q], bf16, kind="Internal")

        for b in range(BPC):
            # ---- setup: load both sides, split, flatten via DRAM ----
            natxy = nat_pool.tile([128, 2 * Q * D], f32, tag="natxy")
            nc.sync.dma_start(
                out=natxy[:, 0 : Q * D],
                in_=x[b].rearrange("(p q) d -> p (q d)", p=128),
            )
            nc.sync.dma_start(
                out=natxy[:, Q * D : 2 * Q * D],
                in_=y[b].rearrange("(p q) d -> p (q d)", p=128),
            )
            # [p, g, d, q] strided view
            natv = natxy.rearrange("p (g q d) -> p g d q", g=2, d=D)

            stxy = stage_pool.tile([128, 2 * K * Q], bf16, tag="stxy")
            stv = stxy.rearrange("p (g f q) -> p g f q", g=2, f=K)
            # x fields: [h0,h1,h2, h0,h1,h2, m0,m1,m2, nh,nm, 1,1]
            # y fields: [h0,h1,h2, m0,m1,m2, h0,h1,h2, 1,1, nh,nm]
            # h main: both sides fields 0-2 in one op
            nc.vector.tensor_copy(stv[:, :, 0:3, :], natv)
            # h dup: x fields 3-5, y fields 6-8 (idle gpsimd)
            nc.scalar.copy(stv[:, 0, 3:6, :], natv[:, 0])
            nc.scalar.copy(stv[:, 1, 6:9, :], natv[:, 1])
            # residual m = nat - h (both sides, one op)
            tmp = nat_pool.tile([128, 2 * Q * D], f32, tag="tmp")
            tmpv = tmp.rearrange("p (g d q) -> p g d q", g=2, q=Q)
            nc.vector.tensor_sub(tmpv, natv, stv[:, :, 0:3, :])
            nc.scalar.copy(stv[:, 0, 6:9, :], tmpv[:, 0])
            nc.scalar.copy(stv[:, 1, 3:6, :], tmpv[:, 1])
            # norms: -0.5*|.|^2, split h+m
            sq = nat_pool.tile([128, 2 * Q * D], f32, tag="sq")
            nc.vector.tensor_mul(sq, natxy, natxy)
            nrm = small_pool.tile([128, 2 * Q], f32, tag="nrm")
            nc.vector.tensor_reduce(
                nrm, sq.rearrange("p (g q d) -> p g q d", g=2, d=D), axis=X,
                op=mybir.AluOpType.add,
            )
            nc.vector.tensor_scalar_mul(nrm, nrm, -0.5)
            nrmv = nrm.rearrange("p (g q) -> p g q", g=2)
            nhx, nhy = stv[:, 0, 9, :], stv[:, 1, 11, :]
            nc.vector.tensor_copy(nhx, nrmv[:, 0])
            nc.vector.tensor_copy(nhy, nrmv[:, 1])
            nrm2 = small_pool.tile([128, 2 * Q], f32, tag="nrm2")
            nrm2v = nrm2.rearrange("p (g q) -> p g q", g=2)
            nc.vector.tensor_sub(nrm2v[:, 0], nrmv[:, 0], nhx)
            nc.vector.tensor_sub(nrm2v[:, 1], nrmv[:, 1], nhy)
            nc.scalar.copy(stv[:, 0, 10, :], nrm2v[:, 0])
            nc.scalar.copy(stv[:, 1, 12, :], nrm2v[:, 1])
            # ones fields
            nc.gpsimd.memset(stxy[:, 11 * Q : 13 * Q], 1.0)
            nc.gpsimd.memset(stxy[:, (K + 9) * Q : (K + 11) * Q], 1.0)

            nc.sync.dma_start(
                out=scratch[b].rearrange("g f p q -> p g f q"),
                in_=stv,
            )
            # aug replicated at partition blocks 0/32/64/96 for PE row tiling
            aug = aug_pool.tile([128, 2 * N], bf16, tag="aug")
            for r in range(4):
                eng = nc.sync if r % 2 == 0 else nc.scalar
                eng.dma_start(
                    out=aug[32 * r : 32 * r + K].rearrange(
                        "f (g p q) -> f g p q", g=2, q=Q
                    ),
                    in_=scratch[b].rearrange("g f p q -> f g p q"),
                )

            # ---- main loop: 16 quads of 4 concurrent matmuls ----
            runmax = run_pool.tile([128, N], bf16, tag="runmax")
            drbuf = dr_pool.tile([128, MT * 1024], bf16, tag="drbuf")
            cp0 = None
            GP_L1 = set()  # gpsimd tensor_tensor unsupported by backend
            for m in range(MT):
                psg = mm_psum.tile([128, 2048], f32, tag="mm")
                for t in range(4):
                    blk = aug[32 * t : 32 * t + K]
                    nc.tensor.matmul(
                        psg[:, t * 512 : (t + 1) * 512],
                        lhsT=blk[:, m * 128 : (m + 1) * 128],
                        rhs=blk[:, N + t * 512 : N + (t + 1) * 512],
                        start=True,
                        stop=True,
                        tile_position=(32 * t, 0),
                    )
                cp = cp_pool.tile([128, 2048], bf16, tag="cp")
                nc.scalar.copy(cp, psg)
                if m == 0:
                    cp0 = cp
                elif m == 1:
                    nc.vector.tensor_max(runmax, cp0, cp)
                else:
                    nc.vector.tensor_max(runmax, runmax, cp)
                # dr level 1: fold column halves
                eng = nc.gpsimd if m in GP_L1 else nc.vector
                eng.tensor_max(
                    drbuf[:, m * 1024 : (m + 1) * 1024],
                    cp[:, 0:1024],
                    cp[:, 1024:2048],
                )

            # ---- dr tree levels 2.., strided in-place across all m ----
            drv = drbuf.rearrange("p (m c) -> p m c", c=1024)
            w = 512
            while w >= 16:
                nc.vector.tensor_max(
                    drv[:, :, 0:w], drv[:, :, 0:w], drv[:, :, w : 2 * w]
                )
                w //= 2
            dr16 = small_pool.tile([128, MT], f32, tag="dr16")
            nc.vector.tensor_reduce(
                dr16, drv[:, :, 0:16], axis=X, op=MAXOP,
            )

            # ---- dl: partition-axis max on the idle gpsimd engine ----
            # quartered so the V dlsum reduces overlap the gpsimd quarters
            from concourse import bass_isa

            dlall = run_pool.tile([128, N], bf16, tag="dlall")
            dlq = small_pool.tile([1, 4], f32, tag="dlq")
            NQ = N // 4
            for qq in range(4):
                sl = slice(qq * NQ, (qq + 1) * NQ)
                nc.gpsimd.partition_all_reduce(
                    dlall[:, sl], runmax[:, sl], channels=128,
                    reduce_op=bass_isa.ReduceOp.max,
                )
                nc.vector.reduce_sum(
                    dlq[0:1, qq : qq + 1], dlall[0:1, sl], axis=X
                )
            dlsum = small_pool.tile([1, 1], f32, tag="dlsum")
            nc.vector.reduce_sum(dlsum, dlq, axis=X)

            # ---- dr partition sum: all-reduce add, then combine ----
            drsum = small_pool.tile([128, 1], f32, tag="drsum")
            nc.vector.reduce_sum(drsum, dr16, axis=X)
            drall = small_pool.tile([128, 1], f32, tag="drall")
            nc.gpsimd.partition_all_reduce(
                drall, drsum, channels=128, reduce_op=bass_isa.ReduceOp.add
            )
            tot = small_pool.tile([1, 1], f32, tag="tot")
            nc.vector.tensor_add(tot, dlsum, drall[0:1, :])
            nc.vector.tensor_scalar_mul(out_sb[0:1, b : b + 1], tot, -2.0 / N)

        nc.gpsimd.dma_start(out=o[0:1, 0:BPC], in_=out_sb)

    nc.compile()
    return nc


def _get_nc():
    if "nc" not in _CACHE:
        _CACHE["nc"] = _build()
    return _CACHE["nc"]


def kernel(x: np.ndarray, y: np.ndarray) -> np.ndarray:
    from concourse.bass_utils import run_bass_kernel_spmd

    x = np.ascontiguousarray(np.asarray(x, dtype=np.float32))
    y = np.ascontiguousarray(np.asarray(y, dtype=np.float32))
    nc = _get_nc()
    in_maps = [
        {"x": x[c * BPC : (c + 1) * BPC], "y": y[c * BPC : (c + 1) * BPC]}
        for c in range(NCORES)
    ]
    res = run_bass_kernel_spmd(nc, in_maps, core_ids=list(range(NCORES)))
    return np.concatenate([r["o"].reshape(BPC) for r in res.results])


# revision 18
# speedup vs baseline: 1.3677x; 1.0347x over previous
"""Chamfer distance loss kernel for Trainium2 (Bass/Tile), 8-core data parallel.

Problem: x, y [16, 2048, 3] fp32. Per batch b:
    P[i,j] = |x_i|^2 + |y_j|^2 - 2 x_i.y_j
    loss[b] = mean_j min_i P[i,j] + mean_i min_j P[i,j]

Strategy (v3):
  - Shard batch dim: 2 batches per core across 8 cores.
  - P = -2*Q with Q[i,j] = x_i.y_j - 0.5|x_i|^2 - 0.5|y_j|^2 as a K=13 bf16
    double-split augmented matmul.  min P == -2 * max Q.
  - PE row tiling: aug operands replicated at partitions 0/32/64/96; each
    m-tile issues 4 concurrent matmuls (tile_position=(32t,0)) covering its
    four 512-wide n-chunks -> one [128,2048] PSUM group, ~3x PE throughput.
  - ACT drains each group to bf16 SBUF (16 wide copies/batch); DVE runs the
    dl running-max chain at 2x and the dr row-max tree (level-batched).
  - dl partition-axis max via 16 XBAR DMA transposes (SP queue) + one
    strided reduce; final means via a DRAM-round-trip partition transpose
    (no PSUM needed outside the matmul groups).
"""

import sys

if "/opt/trn_rl_repo" not in sys.path:
    sys.path.insert(0, "/opt/trn_rl_repo")

import numpy as np

B, N, D = 16, 2048, 3
NCORES = 8
BPC = B // NCORES  # batches per core
MT = N // 128  # 16 m-tiles
Q = N // 128  # 16 points per partition in natural layout
K = 13

_CACHE = {}


def _build():
    from contextlib import ExitStack

    import concourse.bass as bass
    import concourse.mybir as mybir
    import concourse.tile as tile
    from concourse import bacc

    f32 = mybir.dt.float32
    bf16 = mybir.dt.bfloat16

    nc = bacc.Bacc()
    x = nc.dram_tensor("x", [BPC, N, D], f32, kind="ExternalInput")
    y = nc.dram_tensor("y", [BPC, N, D], f32, kind="ExternalInput")
    o = nc.dram_tensor("o", [1, BPC], f32, kind="ExternalOutput")

    X = mybir.AxisListType.X
    MAXOP = mybir.AluOpType.max

    with tile.TileContext(nc) as tc, ExitStack() as ctx:
        singles = ctx.enter_context(tc.tile_pool(name="singles", bufs=1))
        nat_pool = ctx.enter_context(tc.tile_pool(name="nat", bufs=2))
        stage_pool = ctx.enter_context(tc.tile_pool(name="stage", bufs=2))
        aug_pool = ctx.enter_context(tc.tile_pool(name="aug", bufs=2))
        small_pool = ctx.enter_context(tc.tile_pool(name="small", bufs=3))
        run_pool = ctx.enter_context(tc.tile_pool(name="run", bufs=2))
        dr_pool = ctx.enter_context(tc.tile_pool(name="dr", bufs=2))
        cp_pool = ctx.enter_context(tc.tile_pool(name="cp", bufs=4))
        mm_psum = ctx.enter_context(tc.tile_pool(name="mmps", bufs=2, space="PSUM"))

        out_sb = singles.tile([1, BPC], f32)
        scratch = nc.dram_tensor("scratch", [BPC, 2, K, 128, Q], bf16, kind="Internal")

        for b in range(BPC):
            # ---- setup: load both sides, split, flatten via DRAM ----
            natxy = nat_pool.tile([128, 2 * Q * D], f32, tag="natxy")
            nc.sync.dma_start(
                out=natxy[:, 0 : Q * D],
                in_=x[b].rearrange("(p q) d -> p (q d)", p=128),
            )
            nc.sync.dma_start(
                out=natxy[:, Q * D : 2 * Q * D],
                in_=y[b].rearrange("(p q) d -> p (q d)", p=128),
            )
            # [p, g, d, q] strided view
            natv = natxy.rearrange("p (g q d) -> p g d q", g=2, d=D)

            stxy = stage_pool.tile([128, 2 * K * Q], bf16, tag="stxy")
            stv = stxy.rearrange("p (g f q) -> p g f q", g=2, f=K)
            # x fields: [h0,h1,h2, h0,h1,h2, m0,m1,m2, nh,nm, 1,1]
            # y fields: [h0,h1,h2, m0,m1,m2, h0,h1,h2, 1,1, nh,nm]
            # h main: both sides fields 0-2 in one op
            nc.vector.tensor_copy(stv[:, :, 0:3, :], natv)
            # h dup: x fields 3-5, y fields 6-8 (idle gpsimd)
            nc.scalar.copy(stv[:, 0, 3:6, :], natv[:, 0])
            nc.scalar.copy(stv[:, 1, 6:9, :], natv[:, 1])
            # residual m = nat - h (both sides, one op)
            tmp = nat_pool.tile([128, 2 * Q * D], f32, tag="tmp")
            tmpv = tmp.rearrange("p (g d q) -> p g d q", g=2, q=Q)
            nc.vector.tensor_sub(tmpv, natv, stv[:, :, 0:3, :])
            nc.scalar.copy(stv[:, 0, 6:9, :], tmpv[:, 0])
            nc.scalar.copy(stv[:, 1, 3:6, :], tmpv[:, 1])
            # norms: -0.5*|.|^2, split h+m
            sq = nat_pool.tile([128, 2 * Q * D], f32, tag="sq")
            nc.vector.tensor_mul(sq, natxy, natxy)
            nrm = small_pool.tile([128, 2 * Q], f32, tag="nrm")
            nc.vector.tensor_reduce(
                nrm, sq.rearrange("p (g q d) -> p g q d", g=2, d=D), axis=X,
                op=mybir.AluOpType.add,
            )
            nc.vector.tensor_scalar_mul(nrm, nrm, -0.5)
            nrmv = nrm.rearrange("p (g q) -> p g q", g=2)
            nhx, nhy = stv[:, 0, 9, :], stv[:, 1, 11, :]
            nc.vector.tensor_copy(nhx, nrmv[:, 0])
            nc.vector.tensor_copy(nhy, nrmv[:, 1])
            nrm2 = small_pool.tile([128, 2 * Q], f32, tag="nrm2")
            nrm2v = nrm2.rearrange("p (g q) -> p g q", g=2)
            nc.vector.tensor_sub(nrm2v[:, 0], nrmv[:, 0], nhx)
            nc.vector.tensor_sub(nrm2v[:, 1], nrmv[:, 1], nhy)
            nc.scalar.copy(stv[:, 0, 10, :], nrm2v[:, 0])
            nc.scalar.copy(stv[:, 1, 12, :], nrm2v[:, 1])
            # ones fields
            nc.gpsimd.memset(stxy[:, 11 * Q : 13 * Q], 1.0)
            nc.gpsimd.memset(stxy[:, (K + 9) * Q : (K + 11) * Q], 1.0)

            nc.sync.dma_start(
                out=scratch[b].rearrange("g f p q -> p g f q"),
                in_=stv,
            )
            # aug replicated at partition blocks 0/32/64/96 for PE row tiling
            aug = aug_pool.tile([128, 2 * N], bf16, tag="aug")
            for r in range(4):
                eng = nc.sync if r % 2 == 0 else nc.scalar
                eng.dma_start(
                    out=aug[32 * r : 32 * r + K].rearrange(
                        "f (g p q) -> f g p q", g=2, q=Q
                    ),
                    in_=scratch[b].rearrange("g f p q -> f g p q"),
                )

            # ---- main loop: 16 quads of 4 concurrent matmuls ----
            runmax = run_pool.tile([128, N], bf16, tag="runmax")
            drbuf = dr_pool.tile([128, MT * 1024], bf16, tag="drbuf")
            cp0 = None
            GP_L1 = set()  # gpsimd tensor_tensor unsupported by backend
            for m in range(MT):
                psg = mm_psum.tile([128, 2048], f32, tag="mm")
                for t in range(4):
                    blk = aug[32 * t : 32 * t + K]
                    nc.tensor.matmul(
                        psg[:, t * 512 : (t + 1) * 512],
                        lhsT=blk[:, m * 128 : (m + 1) * 128],
                        rhs=blk[:, N + t * 512 : N + (t + 1) * 512],
                        start=True,
                        stop=True,
                        tile_position=(32 * t, 0),
                    )
                cp = cp_pool.tile([128, 2048], bf16, tag="cp")
                nc.scalar.copy(cp, psg)
                if m == 0:
                    cp0 = cp
                elif m == 1:
                    nc.vector.tensor_max(runmax, cp0, cp)
                else:
                    nc.vector.tensor_max(runmax, runmax, cp)
                # dr level 1: fold column halves
                nc.vector.tensor_max(
                    drbuf[:, m * 1024 : (m + 1) * 1024],
                    cp[:, 0:1024],
                    cp[:, 1024:2048],
                )
                # tree levels folded into the loop (chunked, strided)
                drv = drbuf.rearrange("p (mm c) -> p mm c", c=1024)
                if m % 4 == 3:
                    s = slice(m - 3, m + 1)
                    nc.vector.tensor_max(
                        drv[:, s, 0:512], drv[:, s, 0:512], drv[:, s, 512:1024]
                    )
                if m % 8 == 7:
                    s = slice(m - 7, m + 1)
                    nc.vector.tensor_max(
                        drv[:, s, 0:256], drv[:, s, 0:256], drv[:, s, 256:512]
                    )
                    nc.vector.tensor_max(
                        drv[:, s, 0:128], drv[:, s, 0:128], drv[:, s, 128:256]
                    )

            # ---- dr final: one strided reduce over the 128-wide stubs ----
            drv = drbuf.rearrange("p (mm c) -> p mm c", c=1024)
            dr16 = small_pool.tile([128, MT], f32, tag="dr16")
            nc.vector.tensor_reduce(
                dr16, drv[:, :, 0:128], axis=X, op=MAXOP,
            )

            # ---- dl: partition-axis max on the idle gpsimd engine ----
            # quartered; dl sums accumulate on the scalar engine
            from concourse import bass_isa

            dlall = run_pool.tile([128, N], bf16, tag="dlall")
            dlq = small_pool.tile([1, 4], f32, tag="dlq")
            junk = small_pool.tile([1, N // 4], bf16, tag="junk")
            NQ = N // 4
            for qq in range(4):
                sl = slice(qq * NQ, (qq + 1) * NQ)
                nc.gpsimd.partition_all_reduce(
                    dlall[:, sl], runmax[:, sl], channels=128,
                    reduce_op=bass_isa.ReduceOp.max,
                )
                nc.scalar.activation(
                    junk, dlall[0:1, sl],
                    mybir.ActivationFunctionType.Copy,
                    accum_out=dlq[0:1, qq : qq + 1],
                )
            dlsum = small_pool.tile([1, 1], f32, tag="dlsum")
            nc.vector.reduce_sum(dlsum, dlq, axis=X)

            # ---- dr partition sum: all-reduce add, then combine ----
            drsum = small_pool.tile([128, 1], f32, tag="drsum")
            nc.vector.reduce_sum(drsum, dr16, axis=X)
            drall = small_pool.tile([128, 1], f32, tag="drall")
            nc.gpsimd.partition_all_reduce(
                drall, drsum, channels=128, reduce_op=bass_isa.ReduceOp.add
            )
            tot = small_pool.tile([1, 1], f32, tag="tot")
            nc.vector.tensor_add(tot, dlsum, drall[0:1, :])
            nc.vector.tensor_scalar_mul(out_sb[0:1, b : b + 1], tot, -2.0 / N)

        nc.gpsimd.dma_start(out=o[0:1, 0:BPC], in_=out_sb)

    nc.compile()
    return nc


def _get_nc():
    if "nc" not in _CACHE:
        _CACHE["nc"] = _build()
    return _CACHE["nc"]


def kernel(x: np.ndarray, y: np.ndarray) -> np.ndarray:
    from concourse.bass_utils import run_bass_kernel_spmd

    x = np.ascontiguousarray(np.asarray(x, dtype=np.float32))
    y = np.ascontiguousarray(np.asarray(y, dtype=np.float32))
    nc = _get_nc()
    in_maps = [
        {"x": x[c * BPC : (c + 1) * BPC], "y": y[c * BPC : (c + 1) * BPC]}
        for c in range(NCORES)
    ]
    res = run_bass_kernel_spmd(nc, in_maps, core_ids=list(range(NCORES)))
    return np.concatenate([r["o"].reshape(BPC) for r in res.results])
